# revision 65
# baseline (speedup 1.0000x reference)
"""Trainium2 Bass kernel for CausalSelfAttention (GQA, RoPE, prefill).

Tensor-parallel over the 8 query groups: core g owns query heads
[4g, 4g+4) and kv head g.  Each core computes a partial output
(full-shape, f16) that the host sums.

Per-core pipeline (all on one NeuronCore, Tile-scheduled):
  1. qkvT = wqkvT.T @ xT   (f16 matmuls, feature-major out).  Activations
     stream on the SP DMA queue in 4-chunk groups, weights on the ACT DGE
     queue -- two strict-FIFO issue queues, so a slot-blocked issue on one
     stream cannot head-of-line-block the other.
  2. RoPE on q and k (QK scale folded into the exp), v -> token-major via
     PE transposes.
  3. per (batch, head): scores KV-MAJOR (scoresT = kT.T @ qT) into 2-bank
     PSUM tiles (one exp per kv-chunk), exp on ACT straight into the PV
     rhs layout -- no probs transposes.  Causal diagonal masked on GpSimd.
     Row sums ride the expT stream as matmuls with an ALL-ONES stationary
     (every output partition holds the sum = free partition-broadcast).
     Raw y and s evacuate PSUM with plain casts (slots release without
     waiting the normalize); reciprocals are deferred and flushed once per
     batch as an ACT burst (Exp<->Reciprocal table reloads cost 1.28us
     each); the normalize is an all-SBUF f16 multiply (4x DVE mode).
  4. out_partial = yT.T @ wprojT with its own PSUM tag, so its matmuls
     fill PE gaps during the other batch's attention.
"""

import numpy as np

B, T, NE, NH, NQG, HS = 2, 1024, 4096, 32, 8, 128
QPK = NH // NQG          # 4 query heads per kv group
NT = B * T               # 2048 tokens
GW = (QPK + 2) * HS      # 768 qkv rows per group
GQ = QPK * HS            # 512 q cols per group
P = 128
NCORES = 8
KC = NE // P             # 32 contraction chunks for qkv proj
MC = GW // P             # 6 qkv feature chunks
TC8 = T // P             # 8 token chunks per batch
NEG = -1.0e30
SCALE = 1.0 / float(np.sqrt(HS))

_CACHE = {}


def _split_waits(nc, mybir, max_waits=1):
    """walrus in this container rejects >1 sync-wait per instruction;
    hoist extras onto single-wait NoOps just before (equivalent since
    semaphores are monotonic and a sequencer executes in order)."""
    for fn in nc.m.functions:
        for blk in fn.blocks:
            new_list, changed = [], False
            for inst in blk.instructions:
                si = getattr(inst, "sync_info", None)
                if si is not None and len(si.on_wait) > max_waits:
                    waits = list(si.on_wait)
                    for i, w in enumerate(waits[:-max_waits]):
                        nop = mybir.InstNoOp(
                            name=f"{inst.name}-wsplit-{i}", ins=[], outs=[],
                            engine=inst.engine)
                        nop.sync_info = mybir.SyncInfo(on_wait=[w], on_update=[])
                        new_list.append(nop)
                    inst.sync_info = mybir.SyncInfo(
                        on_wait=waits[-max_waits:], on_update=list(si.on_update))
                    changed = True
                new_list.append(inst)
            if changed:
                blk.instructions = new_list


def _build_nc(debug=False, reps=1):
    import concourse.bass as bass
    import concourse.mybir as mybir
    import concourse.tile as tile
    from contextlib import ExitStack

    f32 = mybir.dt.float32
    f32r = mybir.dt.float32r
    f16 = mybir.dt.float16

    nc = bass.Bass()
    xT_d = nc.dram_tensor("xT", [NE, NT], f16, kind="ExternalInput")
    wqkvT_d = nc.dram_tensor("wqkvT", [NE, GW], f16, kind="ExternalInput")
    wprojT_d = nc.dram_tensor("wprojT", [GQ, NE], f16, kind="ExternalInput")
    cc_d = nc.dram_tensor("cc", [P, NT], f32, kind="ExternalInput")
    ss_d = nc.dram_tensor("ss", [P, NT], f32, kind="ExternalInput")
    mask_d = nc.dram_tensor("maskT", [P, P], f16, kind="ExternalInput")
    ones16_d = nc.dram_tensor("ones16", [P, P], f16, kind="ExternalInput")
    ident16_d = nc.dram_tensor("ident16", [P, P], f16, kind="ExternalInput")
    out_d = nc.dram_tensor("out", [NT, NE], f16, kind="ExternalOutput")
    warm_d = nc.dram_tensor("warm", [P, P], f16, kind="ExternalOutput")
    if debug:
        tap_q_d = nc.dram_tensor("tap_q", [P, QPK, NT], f16, kind="ExternalOutput")
        tap_k_d = nc.dram_tensor("tap_k", [P, NT], f16, kind="ExternalOutput")
        tap_v_d = nc.dram_tensor("tap_v", [P, B * TC8, P], f16, kind="ExternalOutput")
        tap_e_d = nc.dram_tensor("tap_e", [P, 4608], f16, kind="ExternalOutput")
        tap_s_d = nc.dram_tensor("tap_s", [P, T], f32, kind="ExternalOutput")
        tap_y_d = nc.dram_tensor("tap_y", [P, QPK, NT], f32, kind="ExternalOutput")

    # column offset of kv-chunk c's block inside the expT tile
    offs, acc = [], 0
    for c in range(TC8):
        offs.append(acc)
        acc += (TC8 - c) * P

    def act_recip(out_ap, in_ap):
        # ACT-engine Reciprocal emitted directly (bass gates it behind an
        # accuracy warning; measured max rel err on this HW is 1.2e-5).
        # Callers must BATCH these away from Exp: each Exp<->Reciprocal
        # switch costs a 1.28us ACT_TABLE_LOAD.
        eng = nc.scalar
        ins = [eng.lower_ap(in_ap)]
        for v in (0.0, 1.0, 0.0):
            ins.append(mybir.ImmediateValue(dtype=mybir.dt.float32, value=v))
        eng.add_instruction(mybir.InstActivation(
            name=nc.get_next_instruction_name(),
            func=mybir.ActivationFunctionType.Reciprocal,
            ins=ins, outs=[eng.lower_ap(out_ap)]))

    with tile.TileContext(nc) as tc:
      for _rep in range(reps):
        sL = ExitStack()   # left-side long-lived pools (y, wp, ob)
        sR = ExitStack()   # right-side pools (qk16, attention-era)
        try:
            # const: 0..~17KB left
            const = sL.enter_context(tc.tile_pool(name="const", bufs=1))
            cc = const.tile([P, NT], f32)
            ss = const.tile([P, NT], f32)
            maskT = const.tile([P, P], f16)
            ones16 = const.tile([P, P], f16)
            ident16 = const.tile([P, P], f16)

            # qk16 on the right: lives through attention
            qk16 = sR.enter_context(tc.tile_pool(name="qk16", bufs=1, side="right"))
            q16 = qk16.tile([P, QPK, NT], f16)
            k16 = qk16.tile([P, NT], f16)
            vtm = qk16.tile([P, B * TC8, P], f16)

            # ============ phase 1+2: qkv projection + rope, per batch ========
            with ExitStack() as sA:
                qkv_pool = sA.enter_context(tc.tile_pool(name="qkv", bufs=1))
                qkv = qkv_pool.tile([P, MC, NT], f16)
                wq_pool = sA.enter_context(tc.tile_pool(name="wq", bufs=1))
                wq = wq_pool.tile([P, KC, GW], f16)
                wqr = wqkvT_d[:].rearrange("(kg c p) m -> p kg c m", p=P, c=4)
                xr = xT_d[:].rearrange("(kg c p) t -> p kg c t", p=P, c=4)
                xs_pool = sA.enter_context(tc.tile_pool(name="xs", bufs=4))
                ps1 = sA.enter_context(
                    tc.tile_pool(name="ps1", bufs=6, space="PSUM"))
                rp = sA.enter_context(tc.tile_pool(name="rope", bufs=2))

                # HAM warm-up: the PE clock sits at 1.2GHz until ~3.4us of
                # sustained activity.  Load ident16 first (32KB, ahead of
                # the weight stream) and run 32 back-to-back transposes on
                # rotating column regions (no WAW between regions, depth-4
                # slot reuse keeps them dense), so real matmuls start at
                # 2.4GHz.  The tail is tapped to a dram output so the chain
                # has a consumer.
                nc.scalar.dma_start(ident16[:], ident16_d[:])
                wt = ps1.tile([P, 512], f16, tag="vt", bufs=2, name="warm")
                for w in range(32):
                    r = (w % 4) * P
                    nc.tensor.transpose(wt[:, r:r + P], ident16[:],
                                        ident16[:])
                wsb = rp.tile([P, P], f16, tag="wsb", name="wsb")
                nc.any.tensor_copy(wsb[:], wt[:, 384:512])
                nc.sync.dma_start(warm_d[:], wsb[:])

                wqg = wq[:].rearrange("p (kg c) m -> p kg c m", c=4)
                for b in range(B):
                    tok = slice(b * T, (b + 1) * T)
                    for n in (2 * b, 2 * b + 1):
                        psums = [ps1.tile([P, 512], f32, tag="ps1",
                                          name=f"ps1_{n}_{m_}")
                                 for m_ in range(MC)]
                        for kg in range(KC // 4):
                            if n == 0:
                                # weight loads on the ACT DGE queue: keeps
                                # the SP queue exclusively for xt so one
                                # slot-blocked issue can't stall the other
                                # stream.  First group split per-chunk so
                                # the first matmul starts ~4us earlier.
                                if kg < 2:
                                    for c4 in range(4):
                                        nc.scalar.dma_start(
                                            wqg[:, kg, c4, :],
                                            wqr[:, kg, c4, :])
                                else:
                                    nc.scalar.dma_start(
                                        wqg[:, kg, :, :], wqr[:, kg, :, :])
                            xt = xs_pool.tile([P, 4, 512], f16, tag="xt",
                                              name=f"xt{n}_{kg}")
                            if n == 0 and kg < 2:
                                for c4 in range(4):
                                    nc.sync.dma_start(
                                        xt[:, c4, :],
                                        xr[:, kg, c4, 0:512])
                            else:
                                nc.sync.dma_start(
                                    xt[:], xr[:, kg, :, n * 512:(n + 1) * 512])
                            for c4 in range(4):
                                k = kg * 4 + c4
                                for m in range(MC):
                                    nc.tensor.matmul(
                                        psums[m][:],
                                        wq[:, k, m * P:(m + 1) * P],
                                        xt[:, c4, :],
                                        start=(k == 0), stop=(k == KC - 1))
                                    if k == KC - 1:
                                        # evac immediately after each m's
                                        # last matmul (split DVE/ACT): the
                                        # slots free while the remaining
                                        # last-k matmuls still stream
                                        if m % 2 == 0:
                                            nc.vector.tensor_copy(
                                                qkv[:, m,
                                                    n * 512:(n + 1) * 512],
                                                psums[m][:])
                                        else:
                                            nc.scalar.copy(
                                                qkv[:, m,
                                                    n * 512:(n + 1) * 512],
                                                psums[m][:])
                        if n == 0:
                            # const loads queue behind n=0's xt stream on SP
                            # (needed first by rope at ~85us; issuing at t=0
                            # would delay the first weight/activation loads)
                            nc.sync.dma_start(cc[:], cc_d[:])
                            nc.sync.dma_start(ss[:], ss_d[:])
                            nc.sync.dma_start(maskT[:], mask_d[:])
                            nc.sync.dma_start(ones16[:], ones16_d[:])
                    # rope for this batch
                    h = HS // 2
                    ccb, ssb = cc[:, tok], ss[:, tok]
                    for hc in range(QPK + 1):
                        src = qkv[:, hc, tok]
                        rot = rp.tile([P, T], f16, tag="rot", name=f"rot{b}_{hc}")
                        nc.sync.dma_start(rot[0:h, :], src[h:P, :])
                        nc.sync.dma_start(rot[h:P, :], src[0:h, :])
                        t1 = rp.tile([P, T], f32, tag="t1", name=f"t1_{b}_{hc}")
                        t2 = rp.tile([P, T], f32, tag="t2", name=f"t2_{b}_{hc}")
                        nc.vector.tensor_mul(t1[:], src, ccb)
                        nc.vector.tensor_mul(t2[:], rot[:], ssb)
                        dst = q16[:, hc, tok] if hc < QPK else k16[:, tok]
                        nc.vector.tensor_add(dst, t1[:], t2[:])
                    for c in range(TC8):
                        # PE transpose (avoids XBAR DMA-transpose, which
                        # races concurrent DMA copies on this stack)
                        vt_ps = ps1.tile([P, P], f16, tag="vt", bufs=2,
                                         name=f"vt{b}_{c}")
                        nc.tensor.transpose(
                            vt_ps[:],
                            qkv[:, QPK + 1, b * T + c * P: b * T + (c + 1) * P],
                            ident16[:])
                        nc.any.tensor_copy(vtm[:, b * TC8 + c, :], vt_ps[:])

            # ============ phases 3+4 pools ============
            # PSUM bank budget (8 banks):
            #   acc (QK scores) bufs=2        -> 2 banks
            #   yps [P,T] f32 bufs=1          -> 2 banks
            #   s   [1,T] f32 bufs=1          -> 2 banks
            #   op  (out-proj psum) bufs=2    -> 2 banks
            # out-proj has its OWN tag so its matmuls can fill PE gaps
            # during attention instead of queueing behind attention's
            # psum-slot sequence.
            y_pool = sL.enter_context(tc.tile_pool(name="y", bufs=1))
            y_sb = y_pool.tile([P, QPK, NT], f16)
            wp_pool = sL.enter_context(tc.tile_pool(name="wp", bufs=1))
            wp = wp_pool.tile([P, QPK, NE], f16)
            wpr = wprojT_d[:].rearrange("(kc p) n -> p kc n", p=P)
            for kc in range(QPK):
                nc.sync.dma_start(wp[:, kc, :], wpr[:, kc, :])
            ob_pool = sL.enter_context(tc.tile_pool(name="ob", bufs=2))

            expT_pool = sR.enter_context(
                tc.tile_pool(name="expT", bufs=3, side="right"))
            stat_pool = sR.enter_context(
                tc.tile_pool(name="stat", bufs=8, side="right"))
            rb_pool = sR.enter_context(
                tc.tile_pool(name="rb", bufs=8, side="right"))
            psA = sR.enter_context(tc.tile_pool(name="psA", bufs=1, space="PSUM"))

            # ============ phase 3: attention ============
            for b in range(B):
                tok = slice(b * T, (b + 1) * T)
                deferred = []
                for hc in range(QPK):
                    qT_i = q16[:, hc, tok]
                    expT = expT_pool.tile([P, acc], f16, tag="expT",
                                          name=f"expT{b}_{hc}")
                    for c in range(TC8):
                        kT_c = k16[:, b * T + c * P: b * T + (c + 1) * P]
                        spans = [(c * P, 512)] if c < 4 else []
                        spans += [(max(512, c * P), T)]
                        # one 2-bank psum tile per kv-chunk: both spans land
                        # in it (each matmul stays within one bank) and a
                        # SINGLE exp covers the whole causal span -- 8 ACT
                        # instructions per head instead of 12
                        sps = psA.tile([P, T], f32, tag="acc", bufs=2,
                                       name=f"sps{b}_{hc}_{c}")
                        for (q0, q1) in spans:
                            nc.tensor.matmul(sps[:, q0:q1], kT_c,
                                             qT_i[:, q0:q1],
                                             start=True, stop=True)
                        nc.scalar.activation(
                            expT[:, offs[c]:offs[c] + (T - c * P)],
                            sps[:, c * P:T],
                            mybir.ActivationFunctionType.Exp, scale=SCALE)
                        # zero the invalid (kv > q) half of the diagonal
                        # block -- on GpSimd (idle engine, SBUF-only op) so
                        # DVE stays clear for the normalize stream
                        nc.gpsimd.tensor_mul(
                            expT[:, offs[c]:offs[c] + P],
                            expT[:, offs[c]:offs[c] + P], maskT[:])
                    # PV + row-sum streams in per-half PSUM tiles (1 bank
                    # each, double-buffered) so each half releases as soon
                    # as its normalize is done.  All-ones stationary means
                    # every partition of sH holds the kv-sum: sum +
                    # partition-broadcast fused into one matmul stream.
                    for (s0, s1) in ((0, 512), (512, T)):
                        ypsH = psA.tile([P, 512], f32, tag="yps", bufs=1,
                                        name=f"yps{b}_{hc}_{s0}")
                        sH = psA.tile([P, 512], f32, tag="s", bufs=1,
                                      name=f"s{b}_{hc}_{s0}")
                        cs = [c for c in range(TC8) if c * P < s1]
                        # all PV first, then all SUM: the SUM stream covers
                        # the y-evac cast latency before the single yps slot
                        # is needed again
                        for c in cs:
                            q0 = max(s0, c * P)
                            sl = slice(offs[c] + (q0 - c * P),
                                       offs[c] + (s1 - c * P))
                            nc.tensor.matmul(
                                ypsH[:, q0 - s0:s1 - s0],
                                vtm[:, b * TC8 + c, :],
                                expT[:, sl], start=(c == 0), stop=(c == cs[-1]))
                        # Evacuate raw y with a cast that depends ONLY on
                        # the PV matmuls (slot frees immediately); the
                        # normalize runs all-SBUF in f16 (4x DVE mode), off
                        # every WAR chain.
                        yraw = rb_pool.tile([P, 512], f16, tag="yraw",
                                            name=f"yraw{b}_{hc}_{s0}")
                        nc.vector.tensor_copy(yraw[:], ypsH[:])
                        # all-ones stationary: every partition of sH holds
                        # the kv-sum (sum + partition-broadcast fused; cost
                        # is N cycles regardless of M)
                        for c in cs:
                            q0 = max(s0, c * P)
                            sl = slice(offs[c] + (q0 - c * P),
                                       offs[c] + (s1 - c * P))
                            nc.tensor.matmul(
                                sH[:, q0 - s0:s1 - s0], ones16[:], expT[:, sl],
                                start=(c == 0), stop=(c == cs[-1]))
                        # evac s to SBUF (releases the PSUM slot; cheap) --
                        # reciprocals are DEFERRED and flushed once per
                        # batch as a back-to-back ACT burst so the Exp<->
                        # Reciprocal table reload (1.28us) is paid once.
                        s16 = stat_pool.tile([P, 512], f16, tag="s16",
                                             name=f"s16_{b}_{hc}_{s0}")
                        nc.vector.tensor_copy(s16[:], sH[:])
                        deferred.append((hc, s0, s1, s16, yraw))
                # flush: batched ACT reciprocals + f16 normalizes (kept as
                # one late emission so most of them bunch on ACT; a
                # tile_critical burst would be thrash-free but globally
                # stalls PE ~8us per flush -- measured net loss)
                for (hc, s0, s1, s16, yraw) in deferred:
                    rb = rb_pool.tile([P, 512], f16, tag="rb",
                                      name=f"rb{b}_{hc}_{s0}")
                    act_recip(rb[:], s16[:])
                    nc.vector.tensor_mul(
                        y_sb[:, hc, b * T + s0:b * T + s1],
                        yraw[:], rb[:])

            if debug:
                nc.sync.dma_start(tap_q_d[:], q16[:])
                nc.sync.dma_start(tap_k_d[:], k16[:])
                nc.sync.dma_start(tap_v_d[:], vtm[:])
                nc.sync.dma_start(tap_y_d[:], y_sb[:].bitcast(f32))

            # ============ phase 4: output projection ============
            for m in range(NT // P):
                ob = ob_pool.tile([P, NE], f16, tag="ob", name=f"ob{m}")
                for n in range(NE // 512):
                    opsum = psA.tile([P, 512], f32, tag="op", bufs=2,
                                     name=f"ops{m}_{n}")
                    for kc in range(QPK):
                        nc.tensor.matmul(
                            opsum[:], y_sb[:, kc, m * P:(m + 1) * P],
                            wp[:, kc, n * 512:(n + 1) * 512],
                            start=(kc == 0), stop=(kc == QPK - 1))
                    nc.any.tensor_copy(ob[:, n * 512:(n + 1) * 512], opsum[:])
                # store per half-row: the kernel's final DMA drains 0.5MB
                # instead of 1MB
                nc.sync.dma_start(out_d[m * P:(m + 1) * P, 0:NE // 2],
                                  ob[:, 0:NE // 2])
                nc.sync.dma_start(out_d[m * P:(m + 1) * P, NE // 2:],
                                  ob[:, NE // 2:])
        finally:
            sR.close()
            sL.close()

    _split_waits(nc, mybir)
    return nc


def _host_prep(x, cos, sin, W_attn, W_proj):
    xT = np.ascontiguousarray(x.reshape(NT, NE).T.astype(np.float16))
    cosT = np.tile(cos.T, (1, B))
    sinT = np.tile(sin.T, (1, B))
    cc = np.ascontiguousarray(
        np.concatenate([cosT, cosT], axis=0), dtype=np.float32)
    ss = np.ascontiguousarray(
        np.concatenate([-sinT, sinT], axis=0), dtype=np.float32)
    # scoresT layout [kv, q]: zero strictly-lower (kv > q) entries post-exp
    maskT = np.triu(np.ones((P, P), dtype=np.float16))
    common = {"xT": xT, "cc": cc, "ss": ss, "maskT": maskT,
              "ones16": np.ones((P, P), dtype=np.float16),
              "ident16": np.eye(P, dtype=np.float16)}
    in_maps = []
    for g in range(NCORES):
        m = dict(common)
        m["wqkvT"] = np.ascontiguousarray(
            W_attn[g * GW:(g + 1) * GW, :].T.astype(np.float16))
        m["wprojT"] = np.ascontiguousarray(
            W_proj[:, g * GQ:(g + 1) * GQ].T.astype(np.float16))
        in_maps.append(m)
    return in_maps


LAST_EXEC_NS = None
LAST_RES = None


def kernel(x, cos, sin, W_attn, W_proj, max_seq_length):
    global LAST_EXEC_NS, LAST_RES
    import os
    from concourse.bass_utils import run_bass_kernel_spmd

    x = np.asarray(x, dtype=np.float32)
    cos = np.asarray(cos, dtype=np.float32)
    sin = np.asarray(sin, dtype=np.float32)
    W_attn = np.asarray(W_attn, dtype=np.float32)
    W_proj = np.asarray(W_proj, dtype=np.float32)

    if "nc" not in _CACHE:
        _CACHE["nc"] = _build_nc()
    nc = _CACHE["nc"]

    in_maps = _host_prep(x, cos, sin, W_attn, W_proj)
    kw = {}
    td = os.environ.get("BASS_KERNEL_TMPDIR")
    if td:
        kw["tmpdir"] = td
    res = run_bass_kernel_spmd(nc, in_maps, core_ids=list(range(NCORES)), **kw)
    LAST_RES = res
    LAST_EXEC_NS = res.exec_time_ns

    acc = res.results[0]["out"].astype(np.float32)
    for g in range(1, NCORES):
        acc = acc + res.results[g]["out"].astype(np.float32)
    return acc.reshape(B, T, NE)



# revision 67
# speedup vs baseline: 1.0153x; 1.0153x over previous
"""Trainium2 Bass kernel for CausalSelfAttention (GQA, RoPE, prefill).

Tensor-parallel over the 8 query groups: core g owns query heads
[4g, 4g+4) and kv head g.  Each core computes a partial output
(full-shape, f16) that the host sums.

Per-core pipeline (all on one NeuronCore, Tile-scheduled):
  1. qkvT = wqkvT.T @ xT   (f16 matmuls, feature-major out).  Activations
     stream on the SP DMA queue in 4-chunk groups, weights on the ACT DGE
     queue -- two strict-FIFO issue queues, so a slot-blocked issue on one
     stream cannot head-of-line-block the other.
  2. RoPE on q and k (QK scale folded into the exp), v -> token-major via
     PE transposes.
  3. per (batch, head): scores KV-MAJOR (scoresT = kT.T @ qT) into 2-bank
     PSUM tiles (one exp per kv-chunk), exp on ACT straight into the PV
     rhs layout -- no probs transposes.  Causal diagonal masked on GpSimd.
     Row sums ride the expT stream as matmuls with an ALL-ONES stationary
     (every output partition holds the sum = free partition-broadcast).
     Raw y and s evacuate PSUM with plain casts (slots release without
     waiting the normalize); reciprocals are deferred and flushed once per
     batch as an ACT burst (Exp<->Reciprocal table reloads cost 1.28us
     each); the normalize is an all-SBUF f16 multiply (4x DVE mode).
  4. out_partial = yT.T @ wprojT with its own PSUM tag, so its matmuls
     fill PE gaps during the other batch's attention.
"""

import numpy as np

B, T, NE, NH, NQG, HS = 2, 1024, 4096, 32, 8, 128
QPK = NH // NQG          # 4 query heads per kv group
NT = B * T               # 2048 tokens
GW = (QPK + 2) * HS      # 768 qkv rows per group
GQ = QPK * HS            # 512 q cols per group
P = 128
NCORES = 8
KC = NE // P             # 32 contraction chunks for qkv proj
MC = GW // P             # 6 qkv feature chunks
TC8 = T // P             # 8 token chunks per batch
NEG = -1.0e30
SCALE = 1.0 / float(np.sqrt(HS))

_CACHE = {}


def _split_waits(nc, mybir, max_waits=1):
    """walrus in this container rejects >1 sync-wait per instruction;
    hoist extras onto single-wait NoOps just before (equivalent since
    semaphores are monotonic and a sequencer executes in order)."""
    for fn in nc.m.functions:
        for blk in fn.blocks:
            new_list, changed = [], False
            for inst in blk.instructions:
                si = getattr(inst, "sync_info", None)
                if si is not None and len(si.on_wait) > max_waits:
                    waits = list(si.on_wait)
                    for i, w in enumerate(waits[:-max_waits]):
                        nop = mybir.InstNoOp(
                            name=f"{inst.name}-wsplit-{i}", ins=[], outs=[],
                            engine=inst.engine)
                        nop.sync_info = mybir.SyncInfo(on_wait=[w], on_update=[])
                        new_list.append(nop)
                    inst.sync_info = mybir.SyncInfo(
                        on_wait=waits[-max_waits:], on_update=list(si.on_update))
                    changed = True
                new_list.append(inst)
            if changed:
                blk.instructions = new_list


def _build_nc(debug=False, reps=1):
    import concourse.bass as bass
    import concourse.mybir as mybir
    import concourse.tile as tile
    from contextlib import ExitStack

    f32 = mybir.dt.float32
    f32r = mybir.dt.float32r
    f16 = mybir.dt.float16

    nc = bass.Bass()
    xT_d = nc.dram_tensor("xT", [NE, NT], f16, kind="ExternalInput")
    wqkvT_d = nc.dram_tensor("wqkvT", [NE, GW], f16, kind="ExternalInput")
    wprojT_d = nc.dram_tensor("wprojT", [GQ, NE], f16, kind="ExternalInput")
    cc_d = nc.dram_tensor("cc", [P, NT], f32, kind="ExternalInput")
    ss_d = nc.dram_tensor("ss", [P, NT], f32, kind="ExternalInput")
    mask_d = nc.dram_tensor("maskT", [P, P], f16, kind="ExternalInput")
    ones16_d = nc.dram_tensor("ones16", [P, P], f16, kind="ExternalInput")
    ident16_d = nc.dram_tensor("ident16", [P, P], f16, kind="ExternalInput")
    out_d = nc.dram_tensor("out", [NT, NE], f16, kind="ExternalOutput")
    warm_d = nc.dram_tensor("warm", [P, P], f16, kind="ExternalOutput")
    if debug:
        tap_q_d = nc.dram_tensor("tap_q", [P, QPK, NT], f16, kind="ExternalOutput")
        tap_k_d = nc.dram_tensor("tap_k", [P, NT], f16, kind="ExternalOutput")
        tap_v_d = nc.dram_tensor("tap_v", [P, B * TC8, P], f16, kind="ExternalOutput")
        tap_e_d = nc.dram_tensor("tap_e", [P, 4608], f16, kind="ExternalOutput")
        tap_s_d = nc.dram_tensor("tap_s", [P, T], f32, kind="ExternalOutput")
        tap_y_d = nc.dram_tensor("tap_y", [P, QPK, NT], f32, kind="ExternalOutput")

    # column offset of kv-chunk c's block inside the expT tile
    offs, acc = [], 0
    for c in range(TC8):
        offs.append(acc)
        acc += (TC8 - c) * P

    def act_recip(out_ap, in_ap):
        # ACT-engine Reciprocal emitted directly (bass gates it behind an
        # accuracy warning; measured max rel err on this HW is 1.2e-5).
        # Callers must BATCH these away from Exp: each Exp<->Reciprocal
        # switch costs a 1.28us ACT_TABLE_LOAD.
        eng = nc.scalar
        ins = [eng.lower_ap(in_ap)]
        for v in (0.0, 1.0, 0.0):
            ins.append(mybir.ImmediateValue(dtype=mybir.dt.float32, value=v))
        eng.add_instruction(mybir.InstActivation(
            name=nc.get_next_instruction_name(),
            func=mybir.ActivationFunctionType.Reciprocal,
            ins=ins, outs=[eng.lower_ap(out_ap)]))

    with tile.TileContext(nc) as tc:
      for _rep in range(reps):
        sL = ExitStack()   # left-side long-lived pools (y, wp, ob)
        sR = ExitStack()   # right-side pools (qk16, attention-era)
        try:
            # const: 0..~17KB left
            const = sL.enter_context(tc.tile_pool(name="const", bufs=1))
            cc = const.tile([P, NT], f32)
            ss = const.tile([P, NT], f32)
            maskT = const.tile([P, P], f16)
            ones16 = const.tile([P, P], f16)
            ident16 = const.tile([P, P], f16)

            # qk16 on the right: lives through attention
            qk16 = sR.enter_context(tc.tile_pool(name="qk16", bufs=1, side="right"))
            q16 = qk16.tile([P, QPK, NT], f16)
            k16 = qk16.tile([P, NT], f16)
            vtm = qk16.tile([P, B * TC8, P], f16)

            # ============ phase 1+2: qkv projection + rope, per batch ========
            with ExitStack() as sA:
                qkv_pool = sA.enter_context(tc.tile_pool(name="qkv", bufs=1))
                qkv = qkv_pool.tile([P, MC, NT], f16)
                wq_pool = sA.enter_context(tc.tile_pool(name="wq", bufs=1))
                wq = wq_pool.tile([P, KC, GW], f16)
                wqr = wqkvT_d[:].rearrange("(kg c p) m -> p kg c m", p=P, c=4)
                xr = xT_d[:].rearrange("(kg c p) t -> p kg c t", p=P, c=4)
                xs_pool = sA.enter_context(tc.tile_pool(name="xs", bufs=4))
                ps1 = sA.enter_context(
                    tc.tile_pool(name="ps1", bufs=6, space="PSUM"))
                rp = sA.enter_context(tc.tile_pool(name="rope", bufs=2))

                # HAM warm-up: the PE clock sits at 1.2GHz until ~3.4us of
                # sustained activity.  Load ident16 first (32KB, ahead of
                # the weight stream) and run 32 back-to-back transposes on
                # rotating column regions (no WAW between regions, depth-4
                # slot reuse keeps them dense), so real matmuls start at
                # 2.4GHz.  The tail is tapped to a dram output so the chain
                # has a consumer.
                nc.scalar.dma_start(ident16[:], ident16_d[:])
                wt = ps1.tile([P, 512], f16, tag="vt", bufs=2, name="warm")
                for w in range(32):
                    r = (w % 4) * P
                    nc.tensor.transpose(wt[:, r:r + P], ident16[:],
                                        ident16[:])
                wsb = rp.tile([P, P], f16, tag="wsb", name="wsb")
                nc.any.tensor_copy(wsb[:], wt[:, 384:512])
                nc.sync.dma_start(warm_d[:], wsb[:])

                wqg = wq[:].rearrange("p (kg c) m -> p kg c m", c=4)
                for b in range(B):
                    tok = slice(b * T, (b + 1) * T)
                    for n in (2 * b, 2 * b + 1):
                        psums = [ps1.tile([P, 512], f32, tag="ps1",
                                          name=f"ps1_{n}_{m_}")
                                 for m_ in range(MC)]
                        for kg in range(KC // 4):
                            if n == 0:
                                # weight loads on the ACT DGE queue: keeps
                                # the SP queue exclusively for xt so one
                                # slot-blocked issue can't stall the other
                                # stream.  First group split per-chunk so
                                # the first matmul starts ~4us earlier.
                                if kg < 2:
                                    for c4 in range(4):
                                        nc.scalar.dma_start(
                                            wqg[:, kg, c4, :],
                                            wqr[:, kg, c4, :])
                                else:
                                    nc.scalar.dma_start(
                                        wqg[:, kg, :, :], wqr[:, kg, :, :])
                            xt = xs_pool.tile([P, 4, 512], f16, tag="xt",
                                              name=f"xt{n}_{kg}")
                            if n == 0 and kg < 2:
                                for c4 in range(4):
                                    nc.sync.dma_start(
                                        xt[:, c4, :],
                                        xr[:, kg, c4, 0:512])
                            else:
                                nc.sync.dma_start(
                                    xt[:], xr[:, kg, :, n * 512:(n + 1) * 512])
                            for c4 in range(4):
                                k = kg * 4 + c4
                                for m in range(MC):
                                    nc.tensor.matmul(
                                        psums[m][:],
                                        wq[:, k, m * P:(m + 1) * P],
                                        xt[:, c4, :],
                                        start=(k == 0), stop=(k == KC - 1))
                                    if k == KC - 1:
                                        # evac immediately after each m's
                                        # last matmul (split DVE/ACT): the
                                        # slots free while the remaining
                                        # last-k matmuls still stream
                                        if m % 2 == 0:
                                            nc.vector.tensor_copy(
                                                qkv[:, m,
                                                    n * 512:(n + 1) * 512],
                                                psums[m][:])
                                        else:
                                            nc.scalar.copy(
                                                qkv[:, m,
                                                    n * 512:(n + 1) * 512],
                                                psums[m][:])
                        if n == 0:
                            # const loads queue behind n=0's xt stream on SP
                            # (needed first by rope at ~85us; issuing at t=0
                            # would delay the first weight/activation loads)
                            nc.sync.dma_start(cc[:], cc_d[:])
                            nc.sync.dma_start(ss[:], ss_d[:])
                            nc.sync.dma_start(maskT[:], mask_d[:])
                            nc.sync.dma_start(ones16[:], ones16_d[:])
                    # rope for this batch
                    h = HS // 2
                    ccb, ssb = cc[:, tok], ss[:, tok]
                    for hc in range(QPK + 1):
                        src = qkv[:, hc, tok]
                        rot = rp.tile([P, T], f16, tag="rot", name=f"rot{b}_{hc}")
                        nc.sync.dma_start(rot[0:h, :], src[h:P, :])
                        nc.sync.dma_start(rot[h:P, :], src[0:h, :])
                        t1 = rp.tile([P, T], f32, tag="t1", name=f"t1_{b}_{hc}")
                        t2 = rp.tile([P, T], f32, tag="t2", name=f"t2_{b}_{hc}")
                        nc.vector.tensor_mul(t1[:], src, ccb)
                        nc.vector.tensor_mul(t2[:], rot[:], ssb)
                        dst = q16[:, hc, tok] if hc < QPK else k16[:, tok]
                        nc.vector.tensor_add(dst, t1[:], t2[:])
                    for c in range(TC8):
                        # PE transpose (avoids XBAR DMA-transpose, which
                        # races concurrent DMA copies on this stack)
                        vt_ps = ps1.tile([P, P], f16, tag="vt", bufs=2,
                                         name=f"vt{b}_{c}")
                        nc.tensor.transpose(
                            vt_ps[:],
                            qkv[:, QPK + 1, b * T + c * P: b * T + (c + 1) * P],
                            ident16[:])
                        nc.any.tensor_copy(vtm[:, b * TC8 + c, :], vt_ps[:])

            # ============ phases 3+4 pools ============
            # PSUM bank budget (8 banks):
            #   acc (QK scores) bufs=2        -> 2 banks
            #   yps [P,T] f32 bufs=1          -> 2 banks
            #   s   [1,T] f32 bufs=1          -> 2 banks
            #   op  (out-proj psum) bufs=2    -> 2 banks
            # out-proj has its OWN tag so its matmuls can fill PE gaps
            # during attention instead of queueing behind attention's
            # psum-slot sequence.
            y_pool = sL.enter_context(tc.tile_pool(name="y", bufs=1))
            y_sb = y_pool.tile([P, QPK, NT], f16)
            wp_pool = sL.enter_context(tc.tile_pool(name="wp", bufs=1))
            wp = wp_pool.tile([P, QPK, NE], f16)
            wpr = wprojT_d[:].rearrange("(kc p) n -> p kc n", p=P)
            for kc in range(QPK):
                nc.sync.dma_start(wp[:, kc, :], wpr[:, kc, :])
            ob_pool = sL.enter_context(tc.tile_pool(name="ob", bufs=2))

            expT_pool = sR.enter_context(
                tc.tile_pool(name="expT", bufs=2, side="right"))
            stat_pool = sR.enter_context(
                tc.tile_pool(name="stat", bufs=8, side="right"))
            rb_pool = sR.enter_context(
                tc.tile_pool(name="rb", bufs=8, side="right"))
            psA = sR.enter_context(tc.tile_pool(name="psA", bufs=1, space="PSUM"))

            # ============ phase 3: attention ============
            for b in range(B):
                tok = slice(b * T, (b + 1) * T)
                deferred = []
                for hc in range(QPK):
                    qT_i = q16[:, hc, tok]
                    expT = expT_pool.tile([P, acc], f16, tag="expT",
                                          name=f"expT{b}_{hc}")
                    for c in range(TC8):
                        kT_c = k16[:, b * T + c * P: b * T + (c + 1) * P]
                        spans = [(c * P, 512)] if c < 4 else []
                        spans += [(max(512, c * P), T)]
                        # one-bank psum tiles, 4-deep: for the late
                        # kv-chunks (single span <=512) this doubles the
                        # QK->exp pipeline depth vs 2x two-bank tiles
                        for (q0, q1) in spans:
                            sps = psA.tile([P, 512], f32, tag="acc", bufs=4,
                                           name=f"sps{b}_{hc}_{c}_{q0}")
                            w = q1 - q0
                            nc.tensor.matmul(sps[:, :w], kT_c,
                                             qT_i[:, q0:q1],
                                             start=True, stop=True)
                            eo = offs[c] + (q0 - c * P)
                            nc.scalar.activation(
                                expT[:, eo:eo + w], sps[:, :w],
                                mybir.ActivationFunctionType.Exp, scale=SCALE)
                        # zero the invalid (kv > q) half of the diagonal
                        # block -- on GpSimd (idle engine, SBUF-only op) so
                        # DVE stays clear for the normalize stream
                        nc.gpsimd.tensor_mul(
                            expT[:, offs[c]:offs[c] + P],
                            expT[:, offs[c]:offs[c] + P], maskT[:])
                    # PV + row-sum streams in per-half PSUM tiles (1 bank
                    # each, double-buffered) so each half releases as soon
                    # as its normalize is done.  All-ones stationary means
                    # every partition of sH holds the kv-sum: sum +
                    # partition-broadcast fused into one matmul stream.
                    for (s0, s1) in ((0, 512), (512, T)):
                        ypsH = psA.tile([P, 512], f32, tag="yps", bufs=1,
                                        name=f"yps{b}_{hc}_{s0}")
                        sH = psA.tile([P, 512], f32, tag="s", bufs=1,
                                      name=f"s{b}_{hc}_{s0}")
                        cs = [c for c in range(TC8) if c * P < s1]
                        # all PV first, then all SUM: the SUM stream covers
                        # the y-evac cast latency before the single yps slot
                        # is needed again
                        for c in cs:
                            q0 = max(s0, c * P)
                            sl = slice(offs[c] + (q0 - c * P),
                                       offs[c] + (s1 - c * P))
                            nc.tensor.matmul(
                                ypsH[:, q0 - s0:s1 - s0],
                                vtm[:, b * TC8 + c, :],
                                expT[:, sl], start=(c == 0), stop=(c == cs[-1]))
                        # Evacuate raw y with a cast that depends ONLY on
                        # the PV matmuls (slot frees immediately); the
                        # normalize runs all-SBUF in f16 (4x DVE mode), off
                        # every WAR chain.
                        yraw = rb_pool.tile([P, 512], f16, tag="yraw",
                                            name=f"yraw{b}_{hc}_{s0}")
                        nc.vector.tensor_copy(yraw[:], ypsH[:])
                        # all-ones stationary: every partition of sH holds
                        # the kv-sum (sum + partition-broadcast fused; cost
                        # is N cycles regardless of M)
                        for c in cs:
                            q0 = max(s0, c * P)
                            sl = slice(offs[c] + (q0 - c * P),
                                       offs[c] + (s1 - c * P))
                            nc.tensor.matmul(
                                sH[:, q0 - s0:s1 - s0], ones16[:], expT[:, sl],
                                start=(c == 0), stop=(c == cs[-1]))
                        # evac s to SBUF (releases the PSUM slot; cheap) --
                        # reciprocals are DEFERRED and flushed once per
                        # batch as a back-to-back ACT burst so the Exp<->
                        # Reciprocal table reload (1.28us) is paid once.
                        s16 = stat_pool.tile([P, 512], f16, tag="s16",
                                             name=f"s16_{b}_{hc}_{s0}")
                        nc.vector.tensor_copy(s16[:], sH[:])
                        deferred.append((hc, s0, s1, s16, yraw))
                # flush: batched ACT reciprocals + f16 normalizes (kept as
                # one late emission so most of them bunch on ACT; a
                # tile_critical burst would be thrash-free but globally
                # stalls PE ~8us per flush -- measured net loss)
                for (hc, s0, s1, s16, yraw) in deferred:
                    rb = rb_pool.tile([P, 512], f16, tag="rb",
                                      name=f"rb{b}_{hc}_{s0}")
                    act_recip(rb[:], s16[:])
                    nc.vector.tensor_mul(
                        y_sb[:, hc, b * T + s0:b * T + s1],
                        yraw[:], rb[:])

            if debug:
                nc.sync.dma_start(tap_q_d[:], q16[:])
                nc.sync.dma_start(tap_k_d[:], k16[:])
                nc.sync.dma_start(tap_v_d[:], vtm[:])
                nc.sync.dma_start(tap_y_d[:], y_sb[:].bitcast(f32))

            # ============ phase 4: output projection ============
            for m in range(NT // P):
                ob = ob_pool.tile([P, NE], f16, tag="ob", name=f"ob{m}")
                for n in range(NE // 512):
                    opsum = psA.tile([P, 512], f32, tag="op", bufs=2,
                                     name=f"ops{m}_{n}")
                    for kc in range(QPK):
                        nc.tensor.matmul(
                            opsum[:], y_sb[:, kc, m * P:(m + 1) * P],
                            wp[:, kc, n * 512:(n + 1) * 512],
                            start=(kc == 0), stop=(kc == QPK - 1))
                    nc.any.tensor_copy(ob[:, n * 512:(n + 1) * 512], opsum[:])
                # store per half-row: the kernel's final DMA drains 0.5MB
                # instead of 1MB
                nc.sync.dma_start(out_d[m * P:(m + 1) * P, 0:NE // 2],
                                  ob[:, 0:NE // 2])
                nc.sync.dma_start(out_d[m * P:(m + 1) * P, NE // 2:],
                                  ob[:, NE // 2:])
        finally:
            sR.close()
            sL.close()

    _split_waits(nc, mybir)
    return nc


def _host_prep(x, cos, sin, W_attn, W_proj):
    xT = np.ascontiguousarray(x.reshape(NT, NE).T.astype(np.float16))
    cosT = np.tile(cos.T, (1, B))
    sinT = np.tile(sin.T, (1, B))
    cc = np.ascontiguousarray(
        np.concatenate([cosT, cosT], axis=0), dtype=np.float32)
    ss = np.ascontiguousarray(
        np.concatenate([-sinT, sinT], axis=0), dtype=np.float32)
    # scoresT layout [kv, q]: zero strictly-lower (kv > q) entries post-exp
    maskT = np.triu(np.ones((P, P), dtype=np.float16))
    common = {"xT": xT, "cc": cc, "ss": ss, "maskT": maskT,
              "ones16": np.ones((P, P), dtype=np.float16),
              "ident16": np.eye(P, dtype=np.float16)}
    in_maps = []
    for g in range(NCORES):
        m = dict(common)
        m["wqkvT"] = np.ascontiguousarray(
            W_attn[g * GW:(g + 1) * GW, :].T.astype(np.float16))
        m["wprojT"] = np.ascontiguousarray(
            W_proj[:, g * GQ:(g + 1) * GQ].T.astype(np.float16))
        in_maps.append(m)
    return in_maps


LAST_EXEC_NS = None
LAST_RES = None


def kernel(x, cos, sin, W_attn, W_proj, max_seq_length):
    global LAST_EXEC_NS, LAST_RES
    import os
    from concourse.bass_utils import run_bass_kernel_spmd

    x = np.asarray(x, dtype=np.float32)
    cos = np.asarray(cos, dtype=np.float32)
    sin = np.asarray(sin, dtype=np.float32)
    W_attn = np.asarray(W_attn, dtype=np.float32)
    W_proj = np.asarray(W_proj, dtype=np.float32)

    if "nc" not in _CACHE:
        _CACHE["nc"] = _build_nc()
    nc = _CACHE["nc"]

    in_maps = _host_prep(x, cos, sin, W_attn, W_proj)
    kw = {}
    td = os.environ.get("BASS_KERNEL_TMPDIR")
    if td:
        kw["tmpdir"] = td
    res = run_bass_kernel_spmd(nc, in_maps, core_ids=list(range(NCORES)), **kw)
    LAST_RES = res
    LAST_EXEC_NS = res.exec_time_ns

    acc = res.results[0]["out"].astype(np.float32)
    for g in range(1, NCORES):
        acc = acc + res.results[g]["out"].astype(np.float32)
    return acc.reshape(B, T, NE)



# revision 69
# speedup vs baseline: 1.0228x; 1.0074x over previous
"""Trainium2 Bass kernel for CausalSelfAttention (GQA, RoPE, prefill).

Tensor-parallel over the 8 query groups: core g owns query heads
[4g, 4g+4) and kv head g.  Each core computes a partial output
(full-shape, f16) that the host sums.

Per-core pipeline (all on one NeuronCore, Tile-scheduled):
  1. qkvT = wqkvT.T @ xT   (f16 matmuls, feature-major out).  Activations
     stream on the SP DMA queue in 4-chunk groups, weights on the ACT DGE
     queue -- two strict-FIFO issue queues, so a slot-blocked issue on one
     stream cannot head-of-line-block the other.
  2. RoPE on q and k (QK scale folded into the exp), v -> token-major via
     PE transposes.
  3. per (batch, head): scores KV-MAJOR (scoresT = kT.T @ qT) into 2-bank
     PSUM tiles (one exp per kv-chunk), exp on ACT straight into the PV
     rhs layout -- no probs transposes.  Causal diagonal masked on GpSimd.
     Row sums ride the expT stream as matmuls with an ALL-ONES stationary
     (every output partition holds the sum = free partition-broadcast).
     Raw y and s evacuate PSUM with plain casts (slots release without
     waiting the normalize); reciprocals are deferred and flushed once per
     batch as an ACT burst (Exp<->Reciprocal table reloads cost 1.28us
     each); the normalize is an all-SBUF f16 multiply (4x DVE mode).
  4. out_partial = yT.T @ wprojT with its own PSUM tag, so its matmuls
     fill PE gaps during the other batch's attention.
"""

import numpy as np

B, T, NE, NH, NQG, HS = 2, 1024, 4096, 32, 8, 128
QPK = NH // NQG          # 4 query heads per kv group
NT = B * T               # 2048 tokens
GW = (QPK + 2) * HS      # 768 qkv rows per group
GQ = QPK * HS            # 512 q cols per group
P = 128
NCORES = 8
KC = NE // P             # 32 contraction chunks for qkv proj
MC = GW // P             # 6 qkv feature chunks
TC8 = T // P             # 8 token chunks per batch
NEG = -1.0e30
SCALE = 1.0 / float(np.sqrt(HS))

_CACHE = {}


def _split_waits(nc, mybir, max_waits=1):
    """walrus in this container rejects >1 sync-wait per instruction;
    hoist extras onto single-wait NoOps just before (equivalent since
    semaphores are monotonic and a sequencer executes in order)."""
    for fn in nc.m.functions:
        for blk in fn.blocks:
            new_list, changed = [], False
            for inst in blk.instructions:
                si = getattr(inst, "sync_info", None)
                if si is not None and len(si.on_wait) > max_waits:
                    waits = list(si.on_wait)
                    for i, w in enumerate(waits[:-max_waits]):
                        nop = mybir.InstNoOp(
                            name=f"{inst.name}-wsplit-{i}", ins=[], outs=[],
                            engine=inst.engine)
                        nop.sync_info = mybir.SyncInfo(on_wait=[w], on_update=[])
                        new_list.append(nop)
                    inst.sync_info = mybir.SyncInfo(
                        on_wait=waits[-max_waits:], on_update=list(si.on_update))
                    changed = True
                new_list.append(inst)
            if changed:
                blk.instructions = new_list


def _build_nc(debug=False, reps=1):
    import concourse.bass as bass
    import concourse.mybir as mybir
    import concourse.tile as tile
    from contextlib import ExitStack

    f32 = mybir.dt.float32
    f32r = mybir.dt.float32r
    f16 = mybir.dt.float16

    nc = bass.Bass()
    xT_d = nc.dram_tensor("xT", [NE, NT], f16, kind="ExternalInput")
    wqkvT_d = nc.dram_tensor("wqkvT", [NE, GW], f16, kind="ExternalInput")
    wprojT_d = nc.dram_tensor("wprojT", [GQ, NE], f16, kind="ExternalInput")
    cc_d = nc.dram_tensor("cc", [P, NT], f32, kind="ExternalInput")
    ss_d = nc.dram_tensor("ss", [P, NT], f32, kind="ExternalInput")
    mask_d = nc.dram_tensor("maskT", [P, P], f16, kind="ExternalInput")
    ones16_d = nc.dram_tensor("ones16", [P, P], f16, kind="ExternalInput")
    ident16_d = nc.dram_tensor("ident16", [P, P], f16, kind="ExternalInput")
    out_d = nc.dram_tensor("out", [NT, NE], f16, kind="ExternalOutput")
    warm_d = nc.dram_tensor("warm", [P, P], f16, kind="ExternalOutput")
    if debug:
        tap_q_d = nc.dram_tensor("tap_q", [P, QPK, NT], f16, kind="ExternalOutput")
        tap_k_d = nc.dram_tensor("tap_k", [P, NT], f16, kind="ExternalOutput")
        tap_v_d = nc.dram_tensor("tap_v", [P, B * TC8, P], f16, kind="ExternalOutput")
        tap_e_d = nc.dram_tensor("tap_e", [P, 4608], f16, kind="ExternalOutput")
        tap_s_d = nc.dram_tensor("tap_s", [P, T], f32, kind="ExternalOutput")
        tap_y_d = nc.dram_tensor("tap_y", [P, QPK, NT], f32, kind="ExternalOutput")

    # column offset of kv-chunk c's block inside the expT tile
    offs, acc = [], 0
    for c in range(TC8):
        offs.append(acc)
        acc += (TC8 - c) * P

    def act_recip(out_ap, in_ap):
        # ACT-engine Reciprocal emitted directly (bass gates it behind an
        # accuracy warning; measured max rel err on this HW is 1.2e-5).
        # Callers must BATCH these away from Exp: each Exp<->Reciprocal
        # switch costs a 1.28us ACT_TABLE_LOAD.
        eng = nc.scalar
        ins = [eng.lower_ap(in_ap)]
        for v in (0.0, 1.0, 0.0):
            ins.append(mybir.ImmediateValue(dtype=mybir.dt.float32, value=v))
        eng.add_instruction(mybir.InstActivation(
            name=nc.get_next_instruction_name(),
            func=mybir.ActivationFunctionType.Reciprocal,
            ins=ins, outs=[eng.lower_ap(out_ap)]))

    with tile.TileContext(nc) as tc:
      for _rep in range(reps):
        sL = ExitStack()   # left-side long-lived pools (y, wp, ob)
        sR = ExitStack()   # right-side pools (qk16, attention-era)
        try:
            # const: 0..~17KB left
            const = sL.enter_context(tc.tile_pool(name="const", bufs=1))
            cc = const.tile([P, NT], f32)
            ss = const.tile([P, NT], f32)
            maskT = const.tile([P, P], f16)
            ones16 = const.tile([P, P], f16)
            ident16 = const.tile([P, P], f16)

            # qk16 on the right: lives through attention
            qk16 = sR.enter_context(tc.tile_pool(name="qk16", bufs=1, side="right"))
            q16 = qk16.tile([P, QPK, NT], f16)
            k16 = qk16.tile([P, NT], f16)
            vtm = qk16.tile([P, B * TC8, P], f16)

            # ============ phase 1+2: qkv projection + rope, per batch ========
            with ExitStack() as sA:
                qkv_pool = sA.enter_context(tc.tile_pool(name="qkv", bufs=1))
                qkv = qkv_pool.tile([P, MC, NT], f16)
                wq_pool = sA.enter_context(tc.tile_pool(name="wq", bufs=1))
                wq = wq_pool.tile([P, KC, GW], f16)
                wqr = wqkvT_d[:].rearrange("(kg c p) m -> p kg c m", p=P, c=4)
                xr = xT_d[:].rearrange("(kg c p) t -> p kg c t", p=P, c=4)
                xs_pool = sA.enter_context(tc.tile_pool(name="xs", bufs=4))
                ps1 = sA.enter_context(
                    tc.tile_pool(name="ps1", bufs=6, space="PSUM"))
                rp = sA.enter_context(tc.tile_pool(name="rope", bufs=2))

                # HAM warm-up: the PE clock sits at 1.2GHz until ~3.4us of
                # sustained activity.  Load ident16 first (32KB, ahead of
                # the weight stream) and run 32 back-to-back transposes on
                # rotating column regions (no WAW between regions, depth-4
                # slot reuse keeps them dense), so real matmuls start at
                # 2.4GHz.  The tail is tapped to a dram output so the chain
                # has a consumer.
                nc.scalar.dma_start(ident16[:], ident16_d[:])
                wt = ps1.tile([P, 512], f16, tag="vt", bufs=2, name="warm")
                for w in range(32):
                    r = (w % 4) * P
                    nc.tensor.transpose(wt[:, r:r + P], ident16[:],
                                        ident16[:])
                wsb = rp.tile([P, P], f16, tag="wsb", name="wsb")
                nc.any.tensor_copy(wsb[:], wt[:, 384:512])
                nc.sync.dma_start(warm_d[:], wsb[:])

                wqg = wq[:].rearrange("p (kg c) m -> p kg c m", c=4)
                for b in range(B):
                    tok = slice(b * T, (b + 1) * T)
                    for n in (2 * b, 2 * b + 1):
                        psums = [ps1.tile([P, 512], f32, tag="ps1",
                                          name=f"ps1_{n}_{m_}")
                                 for m_ in range(MC)]
                        for kg in range(KC // 4):
                            if n == 0:
                                # weight loads on the ACT DGE queue: keeps
                                # the SP queue exclusively for xt so one
                                # slot-blocked issue can't stall the other
                                # stream.  First group split per-chunk so
                                # the first matmul starts ~4us earlier.
                                if kg < 2:
                                    for c4 in range(4):
                                        nc.scalar.dma_start(
                                            wqg[:, kg, c4, :],
                                            wqr[:, kg, c4, :])
                                else:
                                    nc.scalar.dma_start(
                                        wqg[:, kg, :, :], wqr[:, kg, :, :])
                            xt = xs_pool.tile([P, 4, 512], f16, tag="xt",
                                              name=f"xt{n}_{kg}")
                            if n == 0 and kg < 2:
                                for c4 in range(4):
                                    nc.sync.dma_start(
                                        xt[:, c4, :],
                                        xr[:, kg, c4, 0:512])
                            else:
                                nc.sync.dma_start(
                                    xt[:], xr[:, kg, :, n * 512:(n + 1) * 512])
                            for c4 in range(4):
                                k = kg * 4 + c4
                                for m in range(MC):
                                    nc.tensor.matmul(
                                        psums[m][:],
                                        wq[:, k, m * P:(m + 1) * P],
                                        xt[:, c4, :],
                                        start=(k == 0), stop=(k == KC - 1))
                                    if k == KC - 1:
                                        # evac immediately after each m's
                                        # last matmul (split DVE/ACT): the
                                        # slots free while the remaining
                                        # last-k matmuls still stream
                                        if m % 2 == 0:
                                            nc.vector.tensor_copy(
                                                qkv[:, m,
                                                    n * 512:(n + 1) * 512],
                                                psums[m][:])
                                        else:
                                            nc.scalar.copy(
                                                qkv[:, m,
                                                    n * 512:(n + 1) * 512],
                                                psums[m][:])
                        if n == 0:
                            # const loads queue behind n=0's xt stream on SP
                            # (needed first by rope at ~85us; issuing at t=0
                            # would delay the first weight/activation loads)
                            nc.sync.dma_start(cc[:], cc_d[:])
                            nc.sync.dma_start(ss[:], ss_d[:])
                            nc.sync.dma_start(maskT[:], mask_d[:])
                            nc.sync.dma_start(ones16[:], ones16_d[:])
                    # rope for this batch
                    h = HS // 2
                    ccb, ssb = cc[:, tok], ss[:, tok]
                    for hc in range(QPK + 1):
                        src = qkv[:, hc, tok]
                        rot = rp.tile([P, T], f16, tag="rot", name=f"rot{b}_{hc}")
                        nc.sync.dma_start(rot[0:h, :], src[h:P, :])
                        nc.sync.dma_start(rot[h:P, :], src[0:h, :])
                        t1 = rp.tile([P, T], f32, tag="t1", name=f"t1_{b}_{hc}")
                        t2 = rp.tile([P, T], f32, tag="t2", name=f"t2_{b}_{hc}")
                        nc.vector.tensor_mul(t1[:], src, ccb)
                        nc.vector.tensor_mul(t2[:], rot[:], ssb)
                        dst = q16[:, hc, tok] if hc < QPK else k16[:, tok]
                        nc.vector.tensor_add(dst, t1[:], t2[:])
                    for c in range(TC8):
                        # PE transpose (avoids XBAR DMA-transpose, which
                        # races concurrent DMA copies on this stack)
                        vt_ps = ps1.tile([P, P], f16, tag="vt", bufs=2,
                                         name=f"vt{b}_{c}")
                        nc.tensor.transpose(
                            vt_ps[:],
                            qkv[:, QPK + 1, b * T + c * P: b * T + (c + 1) * P],
                            ident16[:])
                        nc.any.tensor_copy(vtm[:, b * TC8 + c, :], vt_ps[:])

            # ============ phases 3+4 pools ============
            # PSUM bank budget (8 banks):
            #   acc (QK scores) bufs=2        -> 2 banks
            #   yps [P,T] f32 bufs=1          -> 2 banks
            #   s   [1,T] f32 bufs=1          -> 2 banks
            #   op  (out-proj psum) bufs=2    -> 2 banks
            # out-proj has its OWN tag so its matmuls can fill PE gaps
            # during attention instead of queueing behind attention's
            # psum-slot sequence.
            y_pool = sL.enter_context(tc.tile_pool(name="y", bufs=1))
            y_sb = y_pool.tile([P, QPK, NT], f16)
            wp_pool = sL.enter_context(tc.tile_pool(name="wp", bufs=1))
            wp = wp_pool.tile([P, QPK, NE], f16)
            wpr = wprojT_d[:].rearrange("(kc p) n -> p kc n", p=P)
            for kc in range(QPK):
                nc.sync.dma_start(wp[:, kc, :], wpr[:, kc, :])
            ob_pool = sL.enter_context(tc.tile_pool(name="ob", bufs=2))

            expT_pool = sR.enter_context(
                tc.tile_pool(name="expT", bufs=2, side="right"))
            stat_pool = sR.enter_context(
                tc.tile_pool(name="stat", bufs=8, side="right"))
            rb_pool = sR.enter_context(
                tc.tile_pool(name="rb", bufs=8, side="right"))
            psA = sR.enter_context(tc.tile_pool(name="psA", bufs=1, space="PSUM"))

            # ============ phase 3: attention ============
            for b in range(B):
                tok = slice(b * T, (b + 1) * T)
                deferred = []
                for hc in range(QPK):
                    qT_i = q16[:, hc, tok]
                    expT = expT_pool.tile([P, acc], f16, tag="expT",
                                          name=f"expT{b}_{hc}")
                    for c in range(TC8):
                        kT_c = k16[:, b * T + c * P: b * T + (c + 1) * P]
                        spans = [(c * P, 512)] if c < 4 else []
                        spans += [(max(512, c * P), T)]
                        # one 2-bank psum tile per kv-chunk: both spans land
                        # in it (each matmul stays within one bank) and a
                        # SINGLE exp covers the whole causal span -- 8 ACT
                        # instructions per head instead of 12
                        sps = psA.tile([P, T], f32, tag="acc", bufs=2,
                                       name=f"sps{b}_{hc}_{c}")
                        for (q0, q1) in spans:
                            nc.tensor.matmul(sps[:, q0:q1], kT_c,
                                             qT_i[:, q0:q1],
                                             start=True, stop=True)
                        nc.scalar.activation(
                            expT[:, offs[c]:offs[c] + (T - c * P)],
                            sps[:, c * P:T],
                            mybir.ActivationFunctionType.Exp, scale=SCALE)
                        # zero the invalid (kv > q) half of the diagonal
                        # block -- on GpSimd (idle engine, SBUF-only op) so
                        # DVE stays clear for the normalize stream
                        nc.gpsimd.tensor_mul(
                            expT[:, offs[c]:offs[c] + P],
                            expT[:, offs[c]:offs[c] + P], maskT[:])
                    # PV + row-sum streams in per-half PSUM tiles (1 bank
                    # each, double-buffered) so each half releases as soon
                    # as its normalize is done.  All-ones stationary means
                    # every partition of sH holds the kv-sum: sum +
                    # partition-broadcast fused into one matmul stream.
                    for (s0, s1) in ((0, 512), (512, T)):
                        ypsH = psA.tile([P, 512], f32, tag="yps", bufs=1,
                                        name=f"yps{b}_{hc}_{s0}")
                        sH = psA.tile([P, 512], f32, tag="s", bufs=1,
                                      name=f"s{b}_{hc}_{s0}")
                        cs = [c for c in range(TC8) if c * P < s1]
                        # all PV first, then all SUM: the SUM stream covers
                        # the y-evac cast latency before the single yps slot
                        # is needed again
                        for c in cs:
                            q0 = max(s0, c * P)
                            sl = slice(offs[c] + (q0 - c * P),
                                       offs[c] + (s1 - c * P))
                            nc.tensor.matmul(
                                ypsH[:, q0 - s0:s1 - s0],
                                vtm[:, b * TC8 + c, :],
                                expT[:, sl], start=(c == 0), stop=(c == cs[-1]))
                        # Evacuate raw y with a cast that depends ONLY on
                        # the PV matmuls (slot frees immediately); the
                        # normalize runs all-SBUF in f16 (4x DVE mode), off
                        # every WAR chain.
                        yraw = rb_pool.tile([P, 512], f16, tag="yraw",
                                            name=f"yraw{b}_{hc}_{s0}")
                        # split across DVE+ACT: this cast is the release op
                        # for the single yps slot, so halving its latency
                        # shortens the next half's PV start
                        nc.vector.tensor_copy(yraw[:, 0:256], ypsH[:, 0:256])
                        nc.scalar.copy(yraw[:, 256:512], ypsH[:, 256:512])
                        # all-ones stationary: every partition of sH holds
                        # the kv-sum (sum + partition-broadcast fused; cost
                        # is N cycles regardless of M)
                        for c in cs:
                            q0 = max(s0, c * P)
                            sl = slice(offs[c] + (q0 - c * P),
                                       offs[c] + (s1 - c * P))
                            nc.tensor.matmul(
                                sH[:, q0 - s0:s1 - s0], ones16[:], expT[:, sl],
                                start=(c == 0), stop=(c == cs[-1]))
                        # evac s to SBUF (releases the PSUM slot; cheap) --
                        # reciprocals are DEFERRED and flushed once per
                        # batch as a back-to-back ACT burst so the Exp<->
                        # Reciprocal table reload (1.28us) is paid once.
                        s16 = stat_pool.tile([P, 512], f16, tag="s16",
                                             name=f"s16_{b}_{hc}_{s0}")
                        nc.vector.tensor_copy(s16[:], sH[:])
                        deferred.append((hc, s0, s1, s16, yraw))
                # flush: batched ACT reciprocals + f16 normalizes (kept as
                # one late emission so most of them bunch on ACT; a
                # tile_critical burst would be thrash-free but globally
                # stalls PE ~8us per flush -- measured net loss)
                for (hc, s0, s1, s16, yraw) in deferred:
                    rb = rb_pool.tile([P, 512], f16, tag="rb",
                                      name=f"rb{b}_{hc}_{s0}")
                    act_recip(rb[:], s16[:])
                    nc.vector.tensor_mul(
                        y_sb[:, hc, b * T + s0:b * T + s1],
                        yraw[:], rb[:])

            if debug:
                nc.sync.dma_start(tap_q_d[:], q16[:])
                nc.sync.dma_start(tap_k_d[:], k16[:])
                nc.sync.dma_start(tap_v_d[:], vtm[:])
                nc.sync.dma_start(tap_y_d[:], y_sb[:].bitcast(f32))

            # ============ phase 4: output projection ============
            for m in range(NT // P):
                ob = ob_pool.tile([P, NE], f16, tag="ob", name=f"ob{m}")
                for n in range(NE // 512):
                    opsum = psA.tile([P, 512], f32, tag="op", bufs=2,
                                     name=f"ops{m}_{n}")
                    for kc in range(QPK):
                        nc.tensor.matmul(
                            opsum[:], y_sb[:, kc, m * P:(m + 1) * P],
                            wp[:, kc, n * 512:(n + 1) * 512],
                            start=(kc == 0), stop=(kc == QPK - 1))
                    nc.any.tensor_copy(ob[:, n * 512:(n + 1) * 512], opsum[:])
                # store per half-row: the kernel's final DMA drains 0.5MB
                # instead of 1MB
                nc.sync.dma_start(out_d[m * P:(m + 1) * P, 0:NE // 2],
                                  ob[:, 0:NE // 2])
                nc.sync.dma_start(out_d[m * P:(m + 1) * P, NE // 2:],
                                  ob[:, NE // 2:])
        finally:
            sR.close()
            sL.close()

    _split_waits(nc, mybir)
    return nc


def _host_prep(x, cos, sin, W_attn, W_proj):
    xT = np.ascontiguousarray(x.reshape(NT, NE).T.astype(np.float16))
    cosT = np.tile(cos.T, (1, B))
    sinT = np.tile(sin.T, (1, B))
    cc = np.ascontiguousarray(
        np.concatenate([cosT, cosT], axis=0), dtype=np.float32)
    ss = np.ascontiguousarray(
        np.concatenate([-sinT, sinT], axis=0), dtype=np.float32)
    # scoresT layout [kv, q]: zero strictly-lower (kv > q) entries post-exp
    maskT = np.triu(np.ones((P, P), dtype=np.float16))
    common = {"xT": xT, "cc": cc, "ss": ss, "maskT": maskT,
              "ones16": np.ones((P, P), dtype=np.float16),
              "ident16": np.eye(P, dtype=np.float16)}
    in_maps = []
    for g in range(NCORES):
        m = dict(common)
        m["wqkvT"] = np.ascontiguousarray(
            W_attn[g * GW:(g + 1) * GW, :].T.astype(np.float16))
        m["wprojT"] = np.ascontiguousarray(
            W_proj[:, g * GQ:(g + 1) * GQ].T.astype(np.float16))
        in_maps.append(m)
    return in_maps


LAST_EXEC_NS = None
LAST_RES = None


def kernel(x, cos, sin, W_attn, W_proj, max_seq_length):
    global LAST_EXEC_NS, LAST_RES
    import os
    from concourse.bass_utils import run_bass_kernel_spmd

    x = np.asarray(x, dtype=np.float32)
    cos = np.asarray(cos, dtype=np.float32)
    sin = np.asarray(sin, dtype=np.float32)
    W_attn = np.asarray(W_attn, dtype=np.float32)
    W_proj = np.asarray(W_proj, dtype=np.float32)

    if "nc" not in _CACHE:
        _CACHE["nc"] = _build_nc()
    nc = _CACHE["nc"]

    in_maps = _host_prep(x, cos, sin, W_attn, W_proj)
    kw = {}
    td = os.environ.get("BASS_KERNEL_TMPDIR")
    if td:
        kw["tmpdir"] = td
    res = run_bass_kernel_spmd(nc, in_maps, core_ids=list(range(NCORES)), **kw)
    LAST_RES = res
    LAST_EXEC_NS = res.exec_time_ns

    acc = res.results[0]["out"].astype(np.float32)
    for g in range(1, NCORES):
        acc = acc + res.results[g]["out"].astype(np.float32)
    return acc.reshape(B, T, NE)



# revision 72
# speedup vs baseline: 1.0386x; 1.0154x over previous
"""Trainium2 Bass kernel for CausalSelfAttention (GQA, RoPE, prefill).

Tensor-parallel over the 8 query groups: core g owns query heads
[4g, 4g+4) and kv head g.  Each core computes a partial output
(full-shape, f16) that the host sums.

Per-core pipeline (all on one NeuronCore, Tile-scheduled):
  1. qkvT = wqkvT.T @ xT   (f16 matmuls, feature-major out).  Activations
     stream on the SP DMA queue in 4-chunk groups, weights on the ACT DGE
     queue -- two strict-FIFO issue queues, so a slot-blocked issue on one
     stream cannot head-of-line-block the other.
  2. RoPE on q and k (QK scale folded into the exp), v -> token-major via
     PE transposes.
  3. per (batch, head): scores KV-MAJOR (scoresT = kT.T @ qT) into 2-bank
     PSUM tiles (one exp per kv-chunk), exp on ACT straight into the PV
     rhs layout -- no probs transposes.  Causal diagonal masked on GpSimd.
     Row sums ride the expT stream as matmuls with an ALL-ONES stationary
     (every output partition holds the sum = free partition-broadcast).
     Raw y and s evacuate PSUM with plain casts (slots release without
     waiting the normalize); reciprocals are deferred and flushed once per
     batch as an ACT burst (Exp<->Reciprocal table reloads cost 1.28us
     each); the normalize is an all-SBUF f16 multiply (4x DVE mode).
  4. out_partial = yT.T @ wprojT with its own PSUM tag, so its matmuls
     fill PE gaps during the other batch's attention.
"""

import numpy as np

B, T, NE, NH, NQG, HS = 2, 1024, 4096, 32, 8, 128
QPK = NH // NQG          # 4 query heads per kv group
NT = B * T               # 2048 tokens
GW = (QPK + 2) * HS      # 768 qkv rows per group
GQ = QPK * HS            # 512 q cols per group
P = 128
NCORES = 8
KC = NE // P             # 32 contraction chunks for qkv proj
MC = GW // P             # 6 qkv feature chunks
TC8 = T // P             # 8 token chunks per batch
NEG = -1.0e30
SCALE = 1.0 / float(np.sqrt(HS))

_CACHE = {}


def _split_waits(nc, mybir, max_waits=1):
    """walrus in this container rejects >1 sync-wait per instruction;
    hoist extras onto single-wait NoOps just before (equivalent since
    semaphores are monotonic and a sequencer executes in order)."""
    for fn in nc.m.functions:
        for blk in fn.blocks:
            new_list, changed = [], False
            for inst in blk.instructions:
                si = getattr(inst, "sync_info", None)
                if si is not None and len(si.on_wait) > max_waits:
                    waits = list(si.on_wait)
                    for i, w in enumerate(waits[:-max_waits]):
                        nop = mybir.InstNoOp(
                            name=f"{inst.name}-wsplit-{i}", ins=[], outs=[],
                            engine=inst.engine)
                        nop.sync_info = mybir.SyncInfo(on_wait=[w], on_update=[])
                        new_list.append(nop)
                    inst.sync_info = mybir.SyncInfo(
                        on_wait=waits[-max_waits:], on_update=list(si.on_update))
                    changed = True
                new_list.append(inst)
            if changed:
                blk.instructions = new_list


def _build_nc(debug=False, reps=1):
    import concourse.bass as bass
    import concourse.mybir as mybir
    import concourse.tile as tile
    from contextlib import ExitStack

    f32 = mybir.dt.float32
    f32r = mybir.dt.float32r
    f16 = mybir.dt.float16

    nc = bass.Bass()
    xT_d = nc.dram_tensor("xT", [NE, NT], f16, kind="ExternalInput")
    wqkvT_d = nc.dram_tensor("wqkvT", [NE, GW], f16, kind="ExternalInput")
    wprojT_d = nc.dram_tensor("wprojT", [GQ, NE], f16, kind="ExternalInput")
    cc_d = nc.dram_tensor("cc", [P, NT], f32, kind="ExternalInput")
    ss_d = nc.dram_tensor("ss", [P, NT], f32, kind="ExternalInput")
    mask_d = nc.dram_tensor("maskT", [P, P], f16, kind="ExternalInput")
    ones16_d = nc.dram_tensor("ones16", [P, P], f16, kind="ExternalInput")
    ident16_d = nc.dram_tensor("ident16", [P, P], f16, kind="ExternalInput")
    out_d = nc.dram_tensor("out", [NT, NE], f16, kind="ExternalOutput")
    warm_d = nc.dram_tensor("warm", [P, P], f16, kind="ExternalOutput")
    if debug:
        tap_q_d = nc.dram_tensor("tap_q", [P, QPK, NT], f16, kind="ExternalOutput")
        tap_k_d = nc.dram_tensor("tap_k", [P, NT], f16, kind="ExternalOutput")
        tap_v_d = nc.dram_tensor("tap_v", [P, B * TC8, P], f16, kind="ExternalOutput")
        tap_e_d = nc.dram_tensor("tap_e", [P, 4608], f16, kind="ExternalOutput")
        tap_s_d = nc.dram_tensor("tap_s", [P, T], f32, kind="ExternalOutput")
        tap_y_d = nc.dram_tensor("tap_y", [P, QPK, NT], f32, kind="ExternalOutput")

    # column offset of kv-chunk c's block inside the expT tile
    offs, acc = [], 0
    for c in range(TC8):
        offs.append(acc)
        acc += (TC8 - c) * P

    def act_recip(out_ap, in_ap):
        # ACT-engine Reciprocal emitted directly (bass gates it behind an
        # accuracy warning; measured max rel err on this HW is 1.2e-5).
        # Callers must BATCH these away from Exp: each Exp<->Reciprocal
        # switch costs a 1.28us ACT_TABLE_LOAD.
        eng = nc.scalar
        ins = [eng.lower_ap(in_ap)]
        for v in (0.0, 1.0, 0.0):
            ins.append(mybir.ImmediateValue(dtype=mybir.dt.float32, value=v))
        eng.add_instruction(mybir.InstActivation(
            name=nc.get_next_instruction_name(),
            func=mybir.ActivationFunctionType.Reciprocal,
            ins=ins, outs=[eng.lower_ap(out_ap)]))

    with tile.TileContext(nc) as tc:
      for _rep in range(reps):
        sL = ExitStack()   # left-side long-lived pools (y, wp, ob)
        sR = ExitStack()   # right-side pools (qk16, attention-era)
        try:
            # const: 0..~17KB left
            const = sL.enter_context(tc.tile_pool(name="const", bufs=1))
            cc = const.tile([P, NT], f32)
            ss = const.tile([P, NT], f32)
            maskT = const.tile([P, P], f16)
            ones16 = const.tile([P, P], f16)
            ident16 = const.tile([P, P], f16)

            # qk16 on the right: lives through attention
            qk16 = sR.enter_context(tc.tile_pool(name="qk16", bufs=1, side="right"))
            q16 = qk16.tile([P, QPK, NT], f16)
            k16 = qk16.tile([P, NT], f16)
            vtm = qk16.tile([P, B * TC8, P], f16)

            # ============ phase 1+2: qkv projection + rope, per batch ========
            with ExitStack() as sA:
                qkv_pool = sA.enter_context(tc.tile_pool(name="qkv", bufs=1))
                qkv = qkv_pool.tile([P, MC, NT], f16)
                wq_pool = sA.enter_context(tc.tile_pool(name="wq", bufs=1))
                wq = wq_pool.tile([P, KC, GW], f16)
                wqr = wqkvT_d[:].rearrange("(kg c p) m -> p kg c m", p=P, c=4)
                xr = xT_d[:].rearrange("(kg c p) t -> p kg c t", p=P, c=4)
                xs_pool = sA.enter_context(tc.tile_pool(name="xs", bufs=4))
                ps1 = sA.enter_context(
                    tc.tile_pool(name="ps1", bufs=6, space="PSUM"))
                rp = sA.enter_context(tc.tile_pool(name="rope", bufs=2))

                # HAM warm-up: the PE clock sits at 1.2GHz until ~3.4us of
                # sustained activity.  Load ident16 first (32KB, ahead of
                # the weight stream) and run 32 back-to-back transposes on
                # rotating column regions (no WAW between regions, depth-4
                # slot reuse keeps them dense), so real matmuls start at
                # 2.4GHz.  The tail is tapped to a dram output so the chain
                # has a consumer.
                nc.scalar.dma_start(ident16[:], ident16_d[:])
                wt = ps1.tile([P, 512], f16, tag="vt", bufs=2, name="warm")
                for w in range(32):
                    r = (w % 4) * P
                    nc.tensor.transpose(wt[:, r:r + P], ident16[:],
                                        ident16[:])
                wsb = rp.tile([P, P], f16, tag="wsb", name="wsb")
                nc.any.tensor_copy(wsb[:], wt[:, 384:512])
                nc.sync.dma_start(warm_d[:], wsb[:])

                wqg = wq[:].rearrange("p (kg c) m -> p kg c m", c=4)
                for b in range(B):
                    tok = slice(b * T, (b + 1) * T)
                    for n in (2 * b, 2 * b + 1):
                        psums = [ps1.tile([P, 512], f32, tag="ps1",
                                          name=f"ps1_{n}_{m_}")
                                 for m_ in range(MC)]
                        for kg in range(KC // 4):
                            if n == 0:
                                # weight loads on the ACT DGE queue: keeps
                                # the SP queue exclusively for xt so one
                                # slot-blocked issue can't stall the other
                                # stream.  First group split per-chunk so
                                # the first matmul starts ~4us earlier.
                                if kg < 2:
                                    for c4 in range(4):
                                        nc.scalar.dma_start(
                                            wqg[:, kg, c4, :],
                                            wqr[:, kg, c4, :])
                                else:
                                    nc.scalar.dma_start(
                                        wqg[:, kg, :, :], wqr[:, kg, :, :])
                            xt = xs_pool.tile([P, 4, 512], f16, tag="xt",
                                              name=f"xt{n}_{kg}")
                            if n == 0 and kg < 2:
                                for c4 in range(4):
                                    nc.sync.dma_start(
                                        xt[:, c4, :],
                                        xr[:, kg, c4, 0:512])
                            else:
                                nc.sync.dma_start(
                                    xt[:], xr[:, kg, :, n * 512:(n + 1) * 512])
                            for c4 in range(4):
                                k = kg * 4 + c4
                                for m in range(MC):
                                    nc.tensor.matmul(
                                        psums[m][:],
                                        wq[:, k, m * P:(m + 1) * P],
                                        xt[:, c4, :],
                                        start=(k == 0), stop=(k == KC - 1))
                                    if k == KC - 1:
                                        # evac immediately after each m's
                                        # last matmul (split DVE/ACT): the
                                        # slots free while the remaining
                                        # last-k matmuls still stream
                                        if m % 2 == 0:
                                            nc.vector.tensor_copy(
                                                qkv[:, m,
                                                    n * 512:(n + 1) * 512],
                                                psums[m][:])
                                        else:
                                            nc.scalar.copy(
                                                qkv[:, m,
                                                    n * 512:(n + 1) * 512],
                                                psums[m][:])
                        if n == 0:
                            # const loads queue behind n=0's xt stream on SP
                            # (needed first by rope at ~85us; issuing at t=0
                            # would delay the first weight/activation loads)
                            nc.sync.dma_start(cc[:], cc_d[:])
                            nc.sync.dma_start(ss[:], ss_d[:])
                            nc.sync.dma_start(maskT[:], mask_d[:])
                            nc.sync.dma_start(ones16[:], ones16_d[:])
                    # rope for this batch
                    h = HS // 2
                    ccb, ssb = cc[:, tok], ss[:, tok]
                    for hc in range(QPK + 1):
                        src = qkv[:, hc, tok]
                        rot = rp.tile([P, T], f16, tag="rot", name=f"rot{b}_{hc}")
                        nc.sync.dma_start(rot[0:h, :], src[h:P, :])
                        nc.sync.dma_start(rot[h:P, :], src[0:h, :])
                        t1 = rp.tile([P, T], f32, tag="t1", name=f"t1_{b}_{hc}")
                        t2 = rp.tile([P, T], f32, tag="t2", name=f"t2_{b}_{hc}")
                        nc.vector.tensor_mul(t1[:], src, ccb)
                        nc.vector.tensor_mul(t2[:], rot[:], ssb)
                        dst = q16[:, hc, tok] if hc < QPK else k16[:, tok]
                        nc.vector.tensor_add(dst, t1[:], t2[:])
                    for c in range(TC8):
                        # PE transpose (avoids XBAR DMA-transpose, which
                        # races concurrent DMA copies on this stack)
                        vt_ps = ps1.tile([P, P], f16, tag="vt", bufs=2,
                                         name=f"vt{b}_{c}")
                        nc.tensor.transpose(
                            vt_ps[:],
                            qkv[:, QPK + 1, b * T + c * P: b * T + (c + 1) * P],
                            ident16[:])
                        nc.any.tensor_copy(vtm[:, b * TC8 + c, :], vt_ps[:])

            # ============ phases 3+4 pools ============
            # PSUM bank budget (8 banks):
            #   acc (QK scores) bufs=2        -> 2 banks
            #   yps [P,T] f32 bufs=1          -> 2 banks
            #   s   [1,T] f32 bufs=1          -> 2 banks
            #   op  (out-proj psum) bufs=2    -> 2 banks
            # out-proj has its OWN tag so its matmuls can fill PE gaps
            # during attention instead of queueing behind attention's
            # psum-slot sequence.
            y_pool = sL.enter_context(tc.tile_pool(name="y", bufs=1))
            y_sb = y_pool.tile([P, QPK, NT], f16)
            wp_pool = sL.enter_context(tc.tile_pool(name="wp", bufs=1))
            wp = wp_pool.tile([P, QPK, NE], f16)
            wpr = wprojT_d[:].rearrange("(kc p) n -> p kc n", p=P)
            for kc in range(QPK):
                nc.sync.dma_start(wp[:, kc, :], wpr[:, kc, :])
            ob_pool = sL.enter_context(tc.tile_pool(name="ob", bufs=2))

            expT_pool = sR.enter_context(
                tc.tile_pool(name="expT", bufs=2, side="right"))
            stat_pool = sR.enter_context(
                tc.tile_pool(name="stat", bufs=8, side="right"))
            rb_pool = sR.enter_context(
                tc.tile_pool(name="rb", bufs=8, side="right"))
            psA = sR.enter_context(tc.tile_pool(name="psA", bufs=1, space="PSUM"))

            # ============ phase 3: attention ============
            for b in range(B):
                tok = slice(b * T, (b + 1) * T)
                deferred = []
                for hc in range(QPK):
                    qT_i = q16[:, hc, tok]
                    expT = expT_pool.tile([P, acc], f16, tag="expT",
                                          name=f"expT{b}_{hc}")
                    for c in range(TC8):
                        kT_c = k16[:, b * T + c * P: b * T + (c + 1) * P]
                        spans = [(c * P, 512)] if c < 4 else []
                        spans += [(max(512, c * P), T)]
                        # one 2-bank psum tile per kv-chunk: both spans land
                        # in it (each matmul stays within one bank) and a
                        # SINGLE exp covers the whole causal span -- 8 ACT
                        # instructions per head instead of 12
                        sps = psA.tile([P, T], f32, tag="acc", bufs=2,
                                       name=f"sps{b}_{hc}_{c}")
                        for (q0, q1) in spans:
                            nc.tensor.matmul(sps[:, q0:q1], kT_c,
                                             qT_i[:, q0:q1],
                                             start=True, stop=True)
                        nc.scalar.activation(
                            expT[:, offs[c]:offs[c] + (T - c * P)],
                            sps[:, c * P:T],
                            mybir.ActivationFunctionType.Exp, scale=SCALE)
                        # zero the invalid (kv > q) half of the diagonal
                        # block -- on GpSimd (idle engine, SBUF-only op) so
                        # DVE stays clear for the normalize stream
                        nc.gpsimd.tensor_mul(
                            expT[:, offs[c]:offs[c] + P],
                            expT[:, offs[c]:offs[c] + P], maskT[:])
                    # PV + row-sum streams in per-half PSUM tiles (1 bank
                    # each, double-buffered) so each half releases as soon
                    # as its normalize is done.  All-ones stationary means
                    # every partition of sH holds the kv-sum: sum +
                    # partition-broadcast fused into one matmul stream.
                    for (s0, s1) in ((0, 512), (512, T)):
                        ypsH = psA.tile([P, 512], f32, tag="yps", bufs=1,
                                        name=f"yps{b}_{hc}_{s0}")
                        sH = psA.tile([P, 512], f32, tag="s", bufs=1,
                                      name=f"s{b}_{hc}_{s0}")
                        cs = [c for c in range(TC8) if c * P < s1]
                        # all PV first, then all SUM: the SUM stream covers
                        # the y-evac cast latency before the single yps slot
                        # is needed again
                        for c in cs:
                            q0 = max(s0, c * P)
                            sl = slice(offs[c] + (q0 - c * P),
                                       offs[c] + (s1 - c * P))
                            nc.tensor.matmul(
                                ypsH[:, q0 - s0:s1 - s0],
                                vtm[:, b * TC8 + c, :],
                                expT[:, sl], start=(c == 0), stop=(c == cs[-1]))
                        # Evacuate raw y with a cast that depends ONLY on
                        # the PV matmuls (slot frees immediately); the
                        # normalize runs all-SBUF in f16 (4x DVE mode), off
                        # every WAR chain.
                        yraw = rb_pool.tile([P, 512], f16, tag="yraw",
                                            name=f"yraw{b}_{hc}_{s0}")
                        nc.vector.tensor_copy(yraw[:], ypsH[:])
                        # all-ones stationary: every partition of sH holds
                        # the kv-sum (sum + partition-broadcast fused; cost
                        # is N cycles regardless of M)
                        for c in cs:
                            q0 = max(s0, c * P)
                            sl = slice(offs[c] + (q0 - c * P),
                                       offs[c] + (s1 - c * P))
                            nc.tensor.matmul(
                                sH[:, q0 - s0:s1 - s0], ones16[:], expT[:, sl],
                                start=(c == 0), stop=(c == cs[-1]))
                        # evac s to SBUF (releases the PSUM slot; cheap) --
                        # reciprocals are DEFERRED and flushed once per
                        # batch as a back-to-back ACT burst so the Exp<->
                        # Reciprocal table reload (1.28us) is paid once.
                        s16 = stat_pool.tile([P, 512], f16, tag="s16",
                                             name=f"s16_{b}_{hc}_{s0}")
                        nc.vector.tensor_copy(s16[:], sH[:])
                        deferred.append((hc, s0, s1, s16, yraw))
                # flush: batched ACT reciprocals + f16 normalizes (kept as
                # one late emission so most of them bunch on ACT; a
                # tile_critical burst would be thrash-free but globally
                # stalls PE ~8us per flush -- measured net loss)
                for (hc, s0, s1, s16, yraw) in deferred:
                    rb = rb_pool.tile([P, 512], f16, tag="rb",
                                      name=f"rb{b}_{hc}_{s0}")
                    act_recip(rb[:], s16[:])
                    nc.vector.tensor_mul(
                        y_sb[:, hc, b * T + s0:b * T + s1],
                        yraw[:], rb[:])

            if debug:
                nc.sync.dma_start(tap_q_d[:], q16[:])
                nc.sync.dma_start(tap_k_d[:], k16[:])
                nc.sync.dma_start(tap_v_d[:], vtm[:])
                nc.sync.dma_start(tap_y_d[:], y_sb[:].bitcast(f32))

            # ============ phase 4: output projection ============
            for m in range(NT // P):
                ob = ob_pool.tile([P, NE], f16, tag="ob", name=f"ob{m}")
                for n in range(NE // 512):
                    opsum = psA.tile([P, 512], f32, tag="op", bufs=2,
                                     name=f"ops{m}_{n}")
                    for kc in range(QPK):
                        nc.tensor.matmul(
                            opsum[:], y_sb[:, kc, m * P:(m + 1) * P],
                            wp[:, kc, n * 512:(n + 1) * 512],
                            start=(kc == 0), stop=(kc == QPK - 1))
                    nc.any.tensor_copy(ob[:, n * 512:(n + 1) * 512], opsum[:])
                # store per half-row: the kernel's final DMA drains 0.5MB
                # instead of 1MB
                nc.sync.dma_start(out_d[m * P:(m + 1) * P, 0:NE // 2],
                                  ob[:, 0:NE // 2])
                nc.sync.dma_start(out_d[m * P:(m + 1) * P, NE // 2:],
                                  ob[:, NE // 2:])
        finally:
            sR.close()
            sL.close()

    _split_waits(nc, mybir)
    return nc


def _host_prep(x, cos, sin, W_attn, W_proj):
    xT = np.ascontiguousarray(x.reshape(NT, NE).T.astype(np.float16))
    cosT = np.tile(cos.T, (1, B))
    sinT = np.tile(sin.T, (1, B))
    cc = np.ascontiguousarray(
        np.concatenate([cosT, cosT], axis=0), dtype=np.float32)
    ss = np.ascontiguousarray(
        np.concatenate([-sinT, sinT], axis=0), dtype=np.float32)
    # scoresT layout [kv, q]: zero strictly-lower (kv > q) entries post-exp
    maskT = np.triu(np.ones((P, P), dtype=np.float16))
    common = {"xT": xT, "cc": cc, "ss": ss, "maskT": maskT,
              "ones16": np.ones((P, P), dtype=np.float16),
              "ident16": np.eye(P, dtype=np.float16)}
    in_maps = []
    for g in range(NCORES):
        m = dict(common)
        m["wqkvT"] = np.ascontiguousarray(
            W_attn[g * GW:(g + 1) * GW, :].T.astype(np.float16))
        m["wprojT"] = np.ascontiguousarray(
            W_proj[:, g * GQ:(g + 1) * GQ].T.astype(np.float16))
        in_maps.append(m)
    return in_maps


LAST_EXEC_NS = None
LAST_RES = None


def kernel(x, cos, sin, W_attn, W_proj, max_seq_length):
    global LAST_EXEC_NS, LAST_RES
    import os
    from concourse.bass_utils import run_bass_kernel_spmd

    x = np.asarray(x, dtype=np.float32)
    cos = np.asarray(cos, dtype=np.float32)
    sin = np.asarray(sin, dtype=np.float32)
    W_attn = np.asarray(W_attn, dtype=np.float32)
    W_proj = np.asarray(W_proj, dtype=np.float32)

    if "nc" not in _CACHE:
        _CACHE["nc"] = _build_nc()
    nc = _CACHE["nc"]

    in_maps = _host_prep(x, cos, sin, W_attn, W_proj)
    kw = {}
    td = os.environ.get("BASS_KERNEL_TMPDIR")
    if td:
        kw["tmpdir"] = td
    res = run_bass_kernel_spmd(nc, in_maps, core_ids=list(range(NCORES)), **kw)
    LAST_RES = res
    LAST_EXEC_NS = res.exec_time_ns

    acc = res.results[0]["out"].astype(np.float32)
    for g in range(1, NCORES):
        acc = acc + res.results[g]["out"].astype(np.float32)
    return acc.reshape(B, T, NE)

